# revision 7
# baseline (speedup 1.0000x reference)
"""DeepSpeed self-attention layer on 8 Trainium2 NeuronCores.

Sharding: tensor-parallel over heads (2 heads/core), DeepSpeed-mp style.
Per core: full x -> layernorm -> transpose -> fused QKV (its 2 heads) ->
rotary -> causal attention (streaming, no-max-softmax with ones-column
row-sum) -> normalized ctx^T -> AllGather -> output projection on this
core's 512-token slice (partition-id dynamic offset) -> host concat.

Host-side folds: norm_w/norm_b into QKV weights/bias, 1/sqrt(HD) into the
q-side rotary tables, input-mask bias into an extra k^T row.
"""

import numpy as np

import concourse.bass as bass
import concourse.mybir as mybir
import concourse.tile as tile
from concourse import bacc
from concourse.masks import make_identity

# Problem shape (hardcoded per contest spec)
B, S, H, NH, HD = 2, 2048, 1024, 16, 64
NCORES = 8
HPC = NH // NCORES          # heads per core = 2
T = B * S                   # 4096 flat tokens
NTILES = T // 128           # 32 token tiles
KC = H // 128               # 8 contraction chunks
TPB = S // 128              # 16 token tiles per batch
GQ = 4                      # q groups of 512 per batch
TPC = T // NCORES           # 512 tokens per core (output slice)
EPS = 1e-5
F32 = mybir.dt.float32


def _bc(ap, count, axis=1):
    """Insert a step-0 broadcast dim of size `count` at free-dim position
    `axis` (1 = right after the partition dim)."""
    new = list(ap.ap)
    new.insert(axis, [0, count])
    return bass.AP(tensor=ap.tensor, offset=ap.offset, ap=new)


def build_nc():
    nc = bacc.Bacc("TRN2", num_devices=NCORES, debug=False)

    x = nc.dram_tensor("x", [T, H], F32, kind="ExternalInput")
    wqkv = nc.dram_tensor("wqkv", [H, 3 * 128], F32, kind="ExternalInput")
    bqkv = nc.dram_tensor("bqkv", [1, 3 * 128], F32, kind="ExternalInput")
    cos_q = nc.dram_tensor("cos_q", [128, TPB, HD], F32, kind="ExternalInput")
    sinx_q = nc.dram_tensor("sinx_q", [128, TPB, HD], F32, kind="ExternalInput")
    cos_k = nc.dram_tensor("cos_k", [128, TPB, HD], F32, kind="ExternalInput")
    sinx_k = nc.dram_tensor("sinx_k", [128, TPB, HD], F32, kind="ExternalInput")
    kbias = nc.dram_tensor("kbias", [B, S], F32, kind="ExternalInput")
    ow = nc.dram_tensor("ow", [H, H], F32, kind="ExternalInput")
    ob = nc.dram_tensor("ob", [1, H], F32, kind="ExternalInput")
    out_slice = nc.dram_tensor("out_slice", [TPC, H], F32, kind="ExternalOutput")

    with tile.TileContext(nc) as tc:
        with (
            tc.tile_pool(name="singles", bufs=1) as singles,
            tc.tile_pool(name="qkvstore", bufs=1) as qkvstore,
            tc.tile_pool(name="dram", bufs=1, space="DRAM") as dram,
        ):
            # ---- constants ----
            ident = singles.tile([128, 128], F32)
            make_identity(nc, ident)
            ones1 = singles.tile([1, 128], F32)
            nc.vector.memset(ones1, 1.0)
            eps_t = singles.tile([128, 1], F32)
            nc.vector.memset(eps_t, EPS)
            wqkv_sb = singles.tile([128, KC, 384], F32)
            nc.sync.dma_start(out=wqkv_sb, in_=wqkv.rearrange("(c p) f -> p c f", p=128))
            bqkv_sb = singles.tile([1, 384], F32)
            nc.sync.dma_start(out=bqkv_sb, in_=bqkv[:, :])
            tabs = {}
            for name, dr in (("cq", cos_q), ("sq", sinx_q), ("ck", cos_k), ("sk", sinx_k)):
                tabs[name] = singles.tile([128, TPB, HD], F32, name=f"tab_{name}", tag=f"tab_{name}")
                nc.sync.dma_start(out=tabs[name], in_=dr[:, :, :])
            ow_sb = singles.tile([128, KC, H], F32)
            nc.sync.dma_start(out=ow_sb, in_=ow.rearrange("(c p) f -> p c f", p=128))
            ob_sb = singles.tile([1, H], F32)
            nc.sync.dma_start(out=ob_sb, in_=ob[:, :])

            # ---- persistent q/k/v storage ----
            qT = qkvstore.tile([65, HPC, T], F32)   # [hd(+ones), head, (b,s)]
            kT = qkvstore.tile([65, HPC, T], F32)   # row 64 = mask bias
            v_sb = qkvstore.tile([128, NTILES, HPC, 65], F32)  # col 64 = 1.0
            nc.vector.memset(qT[64:65, :, :], 1.0)
            nc.vector.memset(v_sb[:, :, :, 64:65], 1.0)
            kb_flat = bass.AP(
                tensor=kbias, offset=0, ap=[[0, 1], [0, HPC], [1, T]]
            )
            nc.sync.dma_start(out=kT[64:65, :, :], in_=kb_flat)

            ctx_local = dram.tile([HPC * HD, T], F32)
            ctx_all = dram.tile([H, T], F32)

            # ================= Phase 1: LN + QKV + rotary =================
            with (
                tc.tile_pool(name="xp", bufs=3) as xp,
                tc.tile_pool(name="xnp", bufs=2) as xnp,
                tc.tile_pool(name="xntp", bufs=2) as xntp,
                tc.tile_pool(name="statp", bufs=4) as statp,
                tc.tile_pool(name="rotp", bufs=3) as rotp,
                tc.tile_pool(name="tp_ps", bufs=3, space="PSUM") as tp_ps,
                tc.tile_pool(name="qkv_ps", bufs=2, space="PSUM") as qkv_ps,
                tc.tile_pool(name="qkt_ps", bufs=3, space="PSUM") as qkt_ps,
            ):
                for t in range(NTILES):
                    st = t % TPB  # position tile within batch
                    x_t = xp.tile([128, H], F32)
                    nc.sync.dma_start(out=x_t, in_=x[t * 128:(t + 1) * 128, :])
                    # layernorm stats
                    stats = statp.tile([128, 2, 6], F32, tag="bnstats")
                    nc.vector.bn_stats(out=stats[:, 0, :], in_=x_t[:, 0:512])
                    nc.vector.bn_stats(out=stats[:, 1, :], in_=x_t[:, 512:1024])
                    mv = statp.tile([128, 2], F32, tag="mv")
                    nc.vector.bn_aggr(out=mv, in_=stats)
                    sq = statp.tile([128, 1], F32, tag="sq")
                    nc.scalar.activation(
                        sq, mv[:, 1:2], mybir.ActivationFunctionType.Sqrt, bias=eps_t[:, 0:1]
                    )
                    rstd = statp.tile([128, 1], F32, tag="rstd")
                    nc.vector.reciprocal(rstd, sq)
                    nmr = statp.tile([128, 1], F32, tag="nmr")
                    nc.vector.tensor_scalar(
                        nmr, mv[:, 0:1], rstd[:, 0:1], -1.0,
                        op0=mybir.AluOpType.mult, op1=mybir.AluOpType.mult,
                    )
                    xn_t = xnp.tile([128, H], F32)
                    nc.scalar.activation(
                        xn_t, x_t, mybir.ActivationFunctionType.Identity,
                        bias=nmr[:, 0:1], scale=rstd[:, 0:1],
                    )
                    # transpose xn -> [feat, tok] chunks
                    xnT = xntp.tile([128, KC, 128], F32)
                    for half in range(2):
                        tp = tp_ps.tile([128, 512], F32, tag="tp")
                        for i in range(4):
                            c = half * 4 + i
                            nc.tensor.transpose(
                                tp[:, i * 128:(i + 1) * 128],
                                xn_t[:, c * 128:(c + 1) * 128], ident,
                            )
                        nc.vector.tensor_copy(
                            xnT[:, half * 4:(half + 1) * 4, :].rearrange("p c f -> p (c f)"),
                            tp,
                        )
                    # fused QKV matmul + rank-1 bias
                    qkvp = qkv_ps.tile([128, 384], F32)
                    for c in range(KC):
                        nc.tensor.matmul(
                            qkvp, xnT[:, c, :], wqkv_sb[:, c, :],
                            start=(c == 0), stop=False,
                        )
                    nc.tensor.matmul(qkvp, ones1, bqkv_sb, start=False, stop=True)

                    # rotary for q and k (tok-orientation, free-dim shifts)
                    for which, (ct, sxt) in (("q", ("cq", "sq")), ("k", ("ck", "sk"))):
                        off = 0 if which == "q" else 128
                        pv = qkvp[:, off:off + 128].rearrange("p (h d) -> p h d", h=HPC)
                        cosb = _bc(tabs[ct][:, st, :], HPC)
                        sinb = _bc(tabs[sxt][:, st, :], HPC)
                        t1 = rotp.tile([128, HPC, HD], F32, tag="t1")
                        nc.vector.tensor_tensor(t1, pv, cosb, op=mybir.AluOpType.mult)
                        qr = rotp.tile([128, HPC, HD], F32, tag="qr")
                        nc.vector.tensor_tensor(
                            qr[:, :, 0:32], pv[:, :, 32:64],
                            bass.AP(tensor=sinb.tensor, offset=sinb.offset, ap=sinb.ap[:2] + [[1, 32]]),
                            op=mybir.AluOpType.mult,
                        )
                        sin_hi = tabs[sxt][:, st, 32:64]
                        nc.vector.tensor_tensor(
                            qr[:, :, 32:64], pv[:, :, 0:32], _bc(sin_hi, HPC),
                            op=mybir.AluOpType.mult,
                        )
                        nc.vector.tensor_tensor(qr, qr, t1, op=mybir.AluOpType.add)
                        dst = qT if which == "q" else kT
                        for h in range(HPC):
                            tph = qkt_ps.tile([64, 128], F32, tag="tph")
                            nc.tensor.transpose(
                                tph, qr[:, h, :], ident
                            )
                            nc.vector.tensor_copy(
                                dst[0:64, h, t * 128:(t + 1) * 128], tph
                            )
                    # v: straight copy from psum
                    nc.vector.tensor_copy(
                        v_sb[:, t, :, 0:64], qkvp[:, 256:384].rearrange("p (h d) -> p h d", h=HPC)
                    )

            # ================= Phase 2: causal attention =================
            with (
                tc.tile_pool(name="pp", bufs=4) as pp,
                tc.tile_pool(name="rp", bufs=2) as rp,
                tc.tile_pool(name="cstp", bufs=3) as cstp,
                tc.tile_pool(name="sc_ps", bufs=4, space="PSUM") as sc_ps,
                tc.tile_pool(name="ctx_ps", bufs=2, space="PSUM") as ctx_ps,
                tc.tile_pool(name="rb_ps", bufs=2, space="PSUM") as rb_ps,
            ):
                for b in range(B):
                    for h in range(HPC):
                        tb = b * S
                        for gq in range(GQ):
                            nkt = 4 * (gq + 1)
                            ctxp = ctx_ps.tile([65, 512], F32)
                            for kt in range(nkt):
                                diag = kt >= 4 * gq
                                qoff = (kt - 4 * gq) * 128 if diag else 0
                                sc = sc_ps.tile([128, 512], F32, tag="sc")
                                nc.tensor.matmul(
                                    sc[:, qoff:512],
                                    kT[:, h, tb + kt * 128: tb + (kt + 1) * 128],
                                    qT[:, h, tb + gq * 512 + qoff: tb + (gq + 1) * 512],
                                    start=True, stop=True,
                                )
                                pb = pp.tile([128, 512], F32, tag="pb")
                                nc.scalar.activation(
                                    pb[:, qoff:512], sc[:, qoff:512],
                                    mybir.ActivationFunctionType.Exp,
                                )
                                if diag:
                                    nc.gpsimd.affine_select(
                                        out=pb[:, qoff:qoff + 128],
                                        in_=pb[:, qoff:qoff + 128],
                                        compare_op=mybir.AluOpType.is_ge,
                                        fill=0.0, base=0,
                                        pattern=[[1, 128]], channel_multiplier=-1,
                                    )
                                nc.tensor.matmul(
                                    ctxp[:, qoff:512],
                                    v_sb[:, b * TPB + kt, h, :],
                                    pb[:, qoff:512],
                                    start=(kt == 0), stop=(kt == nkt - 1),
                                )
                            rin = rp.tile([1, 512], F32, tag="rin")
                            nc.vector.reciprocal(rin, ctxp[64:65, :])
                            rbc = rb_ps.tile([64, 512], F32, tag="rbc")
                            nc.tensor.matmul(
                                rbc, ones1[:, 0:64], rin, start=True, stop=True
                            )
                            cst = cstp.tile([64, 512], F32, tag="cst")
                            nc.vector.tensor_copy(cst, ctxp[0:64, :])
                            nc.vector.tensor_mul(cst, cst, rbc)
                            nc.sync.dma_start(
                                out=ctx_local[h * 64:(h + 1) * 64,
                                              tb + gq * 512: tb + (gq + 1) * 512],
                                in_=cst,
                            )

            # ================= Phase 2.5: AllGather =================
            nc.gpsimd.collective_compute(
                "AllGather", mybir.AluOpType.bypass,
                replica_groups=[list(range(NCORES))],
                ins=[ctx_local.opt()], outs=[ctx_all.opt()],
            )

            # ================= Phase 3: output projection =================
            with (
                tc.tile_pool(name="cap", bufs=2) as cap,
                tc.tile_pool(name="ostg", bufs=3) as ostg,
                tc.tile_pool(name="op_ps", bufs=4, space="PSUM") as op_ps,
            ):
                pid = nc.partition_id()
                base = pid * TPC
                ctx_r = ctx_all.rearrange("(c p) t -> p c t", p=128)
                for tt in range(TPC // 128):
                    ctxA = cap.tile([128, KC, 128], F32)
                    nc.gpsimd.dma_start(
                        out=ctxA, in_=ctx_r[:, :, bass.ds(base + tt * 128, 128)]
                    )
                    for nh in range(2):
                        op = op_ps.tile([128, 512], F32, tag="op")
                        for c in range(KC):
                            nc.tensor.matmul(
                                op, ctxA[:, c, :], ow_sb[:, c, nh * 512:(nh + 1) * 512],
                                start=(c == 0), stop=False,
                            )
                        nc.tensor.matmul(
                            op, ones1, ob_sb[:, nh * 512:(nh + 1) * 512],
                            start=False, stop=True,
                        )
                        ost = ostg.tile([128, 512], F32, tag="ost")
                        nc.vector.tensor_copy(ost, op)
                        nc.sync.dma_start(
                            out=out_slice[tt * 128:(tt + 1) * 128,
                                          nh * 512:(nh + 1) * 512],
                            in_=ost,
                        )
    nc.compile()
    return nc


def make_inputs(x, input_mask, norm_w, norm_b, attn_qkvw, attn_qkvb, attn_ow, attn_ob):
    """Host preprocessing -> list of per-core input dicts."""
    x = np.asarray(x, np.float32).reshape(T, H)
    input_mask = np.asarray(input_mask)
    norm_w = np.asarray(norm_w, np.float32)
    norm_b = np.asarray(norm_b, np.float32)
    attn_qkvw = np.asarray(attn_qkvw, np.float32)
    attn_qkvb = np.asarray(attn_qkvb, np.float32)
    attn_ow = np.asarray(attn_ow, np.float32)
    attn_ob = np.asarray(attn_ob, np.float32)

    wp = norm_w[:, None] * attn_qkvw                     # fold LN scale
    bp = attn_qkvb + norm_b @ attn_qkvw                  # fold LN shift

    pos = np.arange(S, dtype=np.float32)
    inv_freq = 1.0 / (10000.0 ** (np.arange(0, HD, 2, dtype=np.float32) / HD))
    freqs = pos[:, None] * inv_freq[None, :]             # [S, 32]
    cos_full = np.concatenate([np.cos(freqs)] * 2, -1)   # [S, 64]
    sin_full = np.concatenate([np.sin(freqs)] * 2, -1)
    sinx = sin_full.copy()
    sinx[:, :32] *= -1.0

    def tabify(a):  # [S, 64] -> [128, TPB, 64]
        return np.ascontiguousarray(
            a.reshape(TPB, 128, HD).swapaxes(0, 1).astype(np.float32)
        )

    scale = 1.0 / np.sqrt(HD).astype(np.float32)
    cos_q_t = tabify(cos_full * scale)
    sinx_q_t = tabify(sinx * scale)
    cos_k_t = tabify(cos_full)
    sinx_k_t = tabify(sinx)

    kbias_t = ((1.0 - input_mask.astype(np.float32)) * -10000.0).astype(np.float32)

    in_maps = []
    for c in range(NCORES):
        hs = slice(c * HPC * HD, (c + 1) * HPC * HD)     # this core's 128 cols
        wqkv_c = np.ascontiguousarray(
            np.concatenate([wp[:, hs], wp[:, H:][:, hs], wp[:, 2 * H:][:, hs]], axis=1)
        )
        bqkv_c = np.ascontiguousarray(
            np.concatenate([bp[hs], bp[H:][hs], bp[2 * H:][hs]])[None, :]
        )
        in_maps.append({
            "x": x,
            "wqkv": wqkv_c,
            "bqkv": bqkv_c,
            "cos_q": cos_q_t, "sinx_q": sinx_q_t,
            "cos_k": cos_k_t, "sinx_k": sinx_k_t,
            "kbias": kbias_t,
            "ow": attn_ow,
            "ob": np.ascontiguousarray(attn_ob[None, :]),
        })
    return in_maps


_CACHE = {}


def _get_runner():
    """Build nc once and return a callable(in_maps) -> list of out dicts,
    reusing one jitted shard_map across calls."""
    if "runner" in _CACHE:
        return _CACHE["runner"]
    import jax
    import jax.numpy as jnp
    from jax.sharding import Mesh, PartitionSpec
    from jax.experimental.shard_map import shard_map
    from concourse import bass2jax
    from concourse import mybir as _mybir

    nc = build_nc()
    bass2jax.install_neuronx_cc_hook()

    partition_name = nc.partition_id_tensor.name if nc.partition_id_tensor else None
    in_names, out_names, out_avals = [], [], []
    for alloc in nc.m.functions[0].allocations:
        if not isinstance(_mybir.MemoryLocationSet, type) or not isinstance(alloc, _mybir.MemoryLocationSet):
            continue
        name = alloc.memorylocations[0].name
        if alloc.kind == "ExternalInput":
            if name != partition_name:
                in_names.append(name)
        elif alloc.kind == "ExternalOutput":
            out_names.append(name)
            out_avals.append(
                jax.core.ShapedArray(tuple(alloc.tensor_shape), _mybir.dt.np(alloc.dtype))
            )
    n_params = len(in_names)
    all_names = in_names + out_names
    if partition_name is not None:
        all_names.append(partition_name)

    def _body(*args):
        operands = list(args)
        if partition_name is not None:
            operands.append(bass2jax.partition_id_tensor())
        outs = bass2jax._bass_exec_p.bind(
            *operands,
            out_avals=tuple(out_avals),
            in_names=tuple(all_names),
            out_names=tuple(out_names),
            lowering_input_output_aliases=(),
            sim_require_finite=True,
            sim_require_nnan=True,
            nc=nc,
        )
        return tuple(outs)

    devices = jax.devices()[:NCORES]
    mesh = Mesh(np.asarray(devices), ("core",))
    n_outs = len(out_names)
    in_specs = (PartitionSpec("core"),) * (n_params + n_outs)
    out_specs = (PartitionSpec("core"),) * n_outs
    sharded = jax.jit(
        shard_map(_body, mesh=mesh, in_specs=in_specs, out_specs=out_specs,
                  check_rep=False),
        keep_unused=True,
    )

    def runner(in_maps):
        concat_in = [
            np.concatenate([np.asarray(in_maps[c][nm]) for c in range(NCORES)], axis=0)
            for nm in in_names
        ]
        concat_zeros = [
            np.zeros((NCORES * a.shape[0], *a.shape[1:]), a.dtype) for a in out_avals
        ]
        out_arrs = sharded(*concat_in, *concat_zeros)
        jax.block_until_ready(out_arrs)
        return [
            {nm: np.asarray(out_arrs[i]).reshape(NCORES, *out_avals[i].shape)[c]
             for i, nm in enumerate(out_names)}
            for c in range(NCORES)
        ]

    _CACHE["runner"] = runner
    return runner


def kernel(**inputs) -> np.ndarray:
    in_maps = make_inputs(**inputs)
    runner = _get_runner()
    results = runner(in_maps)
    full = np.concatenate([results[c]["out_slice"] for c in range(NCORES)], axis=0)
    return full.reshape(B, S, H).astype(np.float32)


# revision 8
# speedup vs baseline: 29.6002x; 29.6002x over previous
"""DeepSpeed self-attention layer on 8 Trainium2 NeuronCores.

Sharding: tensor-parallel over heads (2 heads/core), DeepSpeed-mp style.
Per core: full x -> layernorm -> transpose -> fused QKV (its 2 heads) ->
rotary -> causal attention (streaming, no-max-softmax with ones-column
row-sum) -> normalized ctx^T -> AllGather -> output projection on this
core's 512-token slice (partition-id dynamic offset) -> host concat.

Host-side folds: norm_w/norm_b into QKV weights/bias, 1/sqrt(HD) into the
q-side rotary tables, input-mask bias into an extra k^T row.
"""

import numpy as np

import concourse.bass as bass
import concourse.mybir as mybir
import concourse.tile as tile
from concourse import bacc
from concourse.masks import make_identity

# Problem shape (hardcoded per contest spec)
B, S, H, NH, HD = 2, 2048, 1024, 16, 64
NCORES = 8
HPC = NH // NCORES          # heads per core = 2
T = B * S                   # 4096 flat tokens
NTILES = T // 128           # 32 token tiles
KC = H // 128               # 8 contraction chunks
TPB = S // 128              # 16 token tiles per batch
GQ = 4                      # q groups of 512 per batch
TPC = T // NCORES           # 512 tokens per core (output slice)
EPS = 1e-5
F32 = mybir.dt.float32


def _bc(ap, count, axis=1):
    """Insert a step-0 broadcast dim of size `count` at free-dim position
    `axis` (1 = right after the partition dim)."""
    new = list(ap.ap)
    new.insert(axis, [0, count])
    return bass.AP(tensor=ap.tensor, offset=ap.offset, ap=new)


def build_nc():
    nc = bacc.Bacc("TRN2", num_devices=NCORES, debug=False)

    x = nc.dram_tensor("x", [T, H], F32, kind="ExternalInput")
    wqkv = nc.dram_tensor("wqkv", [H, 3 * 128], F32, kind="ExternalInput")
    bqkv = nc.dram_tensor("bqkv", [1, 3 * 128], F32, kind="ExternalInput")
    cos_q = nc.dram_tensor("cos_q", [128, TPB, HD], F32, kind="ExternalInput")
    sinx_q = nc.dram_tensor("sinx_q", [128, TPB, HD], F32, kind="ExternalInput")
    cos_k = nc.dram_tensor("cos_k", [128, TPB, HD], F32, kind="ExternalInput")
    sinx_k = nc.dram_tensor("sinx_k", [128, TPB, HD], F32, kind="ExternalInput")
    kbias = nc.dram_tensor("kbias", [B, S], F32, kind="ExternalInput")
    ow = nc.dram_tensor("ow", [H, H], F32, kind="ExternalInput")
    ob = nc.dram_tensor("ob", [1, H], F32, kind="ExternalInput")
    out_slice = nc.dram_tensor("out_slice", [TPC, H], F32, kind="ExternalOutput")

    with tile.TileContext(nc) as tc:
        with (
            tc.tile_pool(name="singles", bufs=1) as singles,
            tc.tile_pool(name="qkvstore", bufs=1) as qkvstore,
            tc.tile_pool(name="dram", bufs=1, space="DRAM") as dram,
        ):
            # ---- constants ----
            ident = singles.tile([128, 128], F32)
            make_identity(nc, ident)
            ones1 = singles.tile([1, 128], F32)
            nc.vector.memset(ones1, 1.0)
            eps_t = singles.tile([128, 1], F32)
            nc.vector.memset(eps_t, EPS)
            wqkv_sb = singles.tile([128, KC, 384], F32)
            nc.sync.dma_start(out=wqkv_sb, in_=wqkv.rearrange("(c p) f -> p c f", p=128))
            bqkv_sb = singles.tile([1, 384], F32)
            nc.sync.dma_start(out=bqkv_sb, in_=bqkv[:, :])
            tabs = {}
            for name, dr in (("cq", cos_q), ("sq", sinx_q), ("ck", cos_k), ("sk", sinx_k)):
                tabs[name] = singles.tile([128, TPB, HD], F32, name=f"tab_{name}", tag=f"tab_{name}")
                nc.sync.dma_start(out=tabs[name], in_=dr[:, :, :])
            ow_sb = singles.tile([128, KC, H], F32)
            nc.sync.dma_start(out=ow_sb, in_=ow.rearrange("(c p) f -> p c f", p=128))
            ob_sb = singles.tile([1, H], F32)
            nc.sync.dma_start(out=ob_sb, in_=ob[:, :])

            # ---- persistent q/k/v storage ----
            qT = qkvstore.tile([65, HPC, T], F32)   # [hd(+ones), head, (b,s)]
            kT = qkvstore.tile([65, HPC, T], F32)   # row 64 = mask bias
            v_sb = qkvstore.tile([128, NTILES, HPC, 65], F32)  # col 64 = 1.0
            nc.vector.memset(qT[64:65, :, :], 1.0)
            nc.vector.memset(v_sb[:, :, :, 64:65], 1.0)
            kb_flat = bass.AP(
                tensor=kbias, offset=0, ap=[[0, 1], [0, HPC], [1, T]]
            )
            nc.sync.dma_start(out=kT[64:65, :, :], in_=kb_flat)

            ctx_local = dram.tile([HPC * HD, T], F32)
            ctx_all = dram.tile([H, T], F32)

            # ================= Phase 1: LN + QKV + rotary =================
            with (
                tc.tile_pool(name="xp", bufs=3) as xp,
                tc.tile_pool(name="xnp", bufs=2) as xnp,
                tc.tile_pool(name="xntp", bufs=2) as xntp,
                tc.tile_pool(name="statp", bufs=4) as statp,
                tc.tile_pool(name="rotp", bufs=3) as rotp,
                tc.tile_pool(name="tp_ps", bufs=3, space="PSUM") as tp_ps,
                tc.tile_pool(name="qkv_ps", bufs=2, space="PSUM") as qkv_ps,
                tc.tile_pool(name="qkt_ps", bufs=3, space="PSUM") as qkt_ps,
            ):
                for t in range(NTILES):
                    st = t % TPB  # position tile within batch
                    x_t = xp.tile([128, H], F32)
                    nc.sync.dma_start(out=x_t, in_=x[t * 128:(t + 1) * 128, :])
                    # layernorm stats
                    stats = statp.tile([128, 2, 6], F32, tag="bnstats")
                    nc.vector.bn_stats(out=stats[:, 0, :], in_=x_t[:, 0:512])
                    nc.vector.bn_stats(out=stats[:, 1, :], in_=x_t[:, 512:1024])
                    mv = statp.tile([128, 2], F32, tag="mv")
                    nc.vector.bn_aggr(out=mv, in_=stats)
                    sq = statp.tile([128, 1], F32, tag="sq")
                    nc.scalar.activation(
                        sq, mv[:, 1:2], mybir.ActivationFunctionType.Sqrt, bias=eps_t[:, 0:1]
                    )
                    rstd = statp.tile([128, 1], F32, tag="rstd")
                    nc.vector.reciprocal(rstd, sq)
                    nmr = statp.tile([128, 1], F32, tag="nmr")
                    nc.vector.tensor_scalar(
                        nmr, mv[:, 0:1], rstd[:, 0:1], -1.0,
                        op0=mybir.AluOpType.mult, op1=mybir.AluOpType.mult,
                    )
                    xn_t = xnp.tile([128, H], F32)
                    nc.scalar.activation(
                        xn_t, x_t, mybir.ActivationFunctionType.Identity,
                        bias=nmr[:, 0:1], scale=rstd[:, 0:1],
                    )
                    # transpose xn -> [feat, tok] chunks
                    xnT = xntp.tile([128, KC, 128], F32)
                    for half in range(2):
                        tp = tp_ps.tile([128, 512], F32, tag="tp")
                        for i in range(4):
                            c = half * 4 + i
                            nc.tensor.transpose(
                                tp[:, i * 128:(i + 1) * 128],
                                xn_t[:, c * 128:(c + 1) * 128], ident,
                            )
                        nc.vector.tensor_copy(
                            xnT[:, half * 4:(half + 1) * 4, :].rearrange("p c f -> p (c f)"),
                            tp,
                        )
                    # fused QKV matmul + rank-1 bias
                    qkvp = qkv_ps.tile([128, 384], F32)
                    for c in range(KC):
                        nc.tensor.matmul(
                            qkvp, xnT[:, c, :], wqkv_sb[:, c, :],
                            start=(c == 0), stop=False,
                        )
                    nc.tensor.matmul(qkvp, ones1, bqkv_sb, start=False, stop=True)

                    # rotary for q and k (tok-orientation, free-dim shifts)
                    for which, (ct, sxt) in (("q", ("cq", "sq")), ("k", ("ck", "sk"))):
                        off = 0 if which == "q" else 128
                        pv = qkvp[:, off:off + 128].rearrange("p (h d) -> p h d", h=HPC)
                        cosb = _bc(tabs[ct][:, st, :], HPC)
                        sinb = _bc(tabs[sxt][:, st, :], HPC)
                        t1 = rotp.tile([128, HPC, HD], F32, tag="t1")
                        nc.vector.tensor_tensor(t1, pv, cosb, op=mybir.AluOpType.mult)
                        qr = rotp.tile([128, HPC, HD], F32, tag="qr")
                        nc.vector.tensor_tensor(
                            qr[:, :, 0:32], pv[:, :, 32:64],
                            bass.AP(tensor=sinb.tensor, offset=sinb.offset, ap=sinb.ap[:2] + [[1, 32]]),
                            op=mybir.AluOpType.mult,
                        )
                        sin_hi = tabs[sxt][:, st, 32:64]
                        nc.vector.tensor_tensor(
                            qr[:, :, 32:64], pv[:, :, 0:32], _bc(sin_hi, HPC),
                            op=mybir.AluOpType.mult,
                        )
                        nc.vector.tensor_tensor(qr, qr, t1, op=mybir.AluOpType.add)
                        dst = qT if which == "q" else kT
                        for h in range(HPC):
                            tph = qkt_ps.tile([64, 128], F32, tag="tph")
                            nc.tensor.transpose(
                                tph, qr[:, h, :], ident
                            )
                            nc.vector.tensor_copy(
                                dst[0:64, h, t * 128:(t + 1) * 128], tph
                            )
                    # v: straight copy from psum
                    nc.vector.tensor_copy(
                        v_sb[:, t, :, 0:64], qkvp[:, 256:384].rearrange("p (h d) -> p h d", h=HPC)
                    )

            # ================= Phase 2: causal attention =================
            with (
                tc.tile_pool(name="pp", bufs=4) as pp,
                tc.tile_pool(name="rp", bufs=2) as rp,
                tc.tile_pool(name="cstp", bufs=3) as cstp,
                tc.tile_pool(name="sc_ps", bufs=4, space="PSUM") as sc_ps,
                tc.tile_pool(name="ctx_ps", bufs=2, space="PSUM") as ctx_ps,
                tc.tile_pool(name="rb_ps", bufs=2, space="PSUM") as rb_ps,
            ):
                for b in range(B):
                    for h in range(HPC):
                        tb = b * S
                        for gq in range(GQ):
                            nkt = 4 * (gq + 1)
                            ctxp = ctx_ps.tile([65, 512], F32)
                            for kt in range(nkt):
                                diag = kt >= 4 * gq
                                qoff = (kt - 4 * gq) * 128 if diag else 0
                                sc = sc_ps.tile([128, 512], F32, tag="sc")
                                nc.tensor.matmul(
                                    sc[:, qoff:512],
                                    kT[:, h, tb + kt * 128: tb + (kt + 1) * 128],
                                    qT[:, h, tb + gq * 512 + qoff: tb + (gq + 1) * 512],
                                    start=True, stop=True,
                                )
                                pb = pp.tile([128, 512], F32, tag="pb")
                                nc.scalar.activation(
                                    pb[:, qoff:512], sc[:, qoff:512],
                                    mybir.ActivationFunctionType.Exp,
                                )
                                if diag:
                                    nc.gpsimd.affine_select(
                                        out=pb[:, qoff:qoff + 128],
                                        in_=pb[:, qoff:qoff + 128],
                                        compare_op=mybir.AluOpType.is_ge,
                                        fill=0.0, base=0,
                                        pattern=[[1, 128]], channel_multiplier=-1,
                                    )
                                nc.tensor.matmul(
                                    ctxp[:, qoff:512],
                                    v_sb[:, b * TPB + kt, h, :],
                                    pb[:, qoff:512],
                                    start=(kt == 0), stop=(kt == nkt - 1),
                                )
                            rin = rp.tile([1, 512], F32, tag="rin")
                            nc.vector.reciprocal(rin, ctxp[64:65, :])
                            rbc = rb_ps.tile([64, 512], F32, tag="rbc")
                            nc.tensor.matmul(
                                rbc, ones1[:, 0:64], rin, start=True, stop=True
                            )
                            cst = cstp.tile([64, 512], F32, tag="cst")
                            nc.vector.tensor_copy(cst, ctxp[0:64, :])
                            nc.vector.tensor_mul(cst, cst, rbc)
                            nc.sync.dma_start(
                                out=ctx_local[h * 64:(h + 1) * 64,
                                              tb + gq * 512: tb + (gq + 1) * 512],
                                in_=cst,
                            )

            # ================= Phase 2.5: AllGather =================
            nc.gpsimd.collective_compute(
                "AllGather", mybir.AluOpType.bypass,
                replica_groups=[list(range(NCORES))],
                ins=[ctx_local.opt()], outs=[ctx_all.opt()],
            )

            # ================= Phase 3: output projection =================
            with (
                tc.tile_pool(name="cap", bufs=2) as cap,
                tc.tile_pool(name="ostg", bufs=3) as ostg,
                tc.tile_pool(name="op_ps", bufs=4, space="PSUM") as op_ps,
            ):
                pid = nc.partition_id()
                base = pid * TPC
                ctx_r = ctx_all.rearrange("(c p) t -> p c t", p=128)
                for tt in range(TPC // 128):
                    ctxA = cap.tile([128, KC, 128], F32)
                    nc.gpsimd.dma_start(
                        out=ctxA, in_=ctx_r[:, :, bass.ds(base + tt * 128, 128)]
                    )
                    for nh in range(2):
                        op = op_ps.tile([128, 512], F32, tag="op")
                        for c in range(KC):
                            nc.tensor.matmul(
                                op, ctxA[:, c, :], ow_sb[:, c, nh * 512:(nh + 1) * 512],
                                start=(c == 0), stop=False,
                            )
                        nc.tensor.matmul(
                            op, ones1, ob_sb[:, nh * 512:(nh + 1) * 512],
                            start=False, stop=True,
                        )
                        ost = ostg.tile([128, 512], F32, tag="ost")
                        nc.vector.tensor_copy(ost, op)
                        nc.sync.dma_start(
                            out=out_slice[tt * 128:(tt + 1) * 128,
                                          nh * 512:(nh + 1) * 512],
                            in_=ost,
                        )
    nc.compile()
    return nc


def make_inputs(x, input_mask, norm_w, norm_b, attn_qkvw, attn_qkvb, attn_ow, attn_ob):
    """Host preprocessing -> list of per-core input dicts."""
    x = np.asarray(x, np.float32).reshape(T, H)
    input_mask = np.asarray(input_mask)
    norm_w = np.asarray(norm_w, np.float32)
    norm_b = np.asarray(norm_b, np.float32)
    attn_qkvw = np.asarray(attn_qkvw, np.float32)
    attn_qkvb = np.asarray(attn_qkvb, np.float32)
    attn_ow = np.asarray(attn_ow, np.float32)
    attn_ob = np.asarray(attn_ob, np.float32)

    wp = norm_w[:, None] * attn_qkvw                     # fold LN scale
    bp = attn_qkvb + norm_b @ attn_qkvw                  # fold LN shift

    pos = np.arange(S, dtype=np.float32)
    inv_freq = 1.0 / (10000.0 ** (np.arange(0, HD, 2, dtype=np.float32) / HD))
    freqs = pos[:, None] * inv_freq[None, :]             # [S, 32]
    cos_full = np.concatenate([np.cos(freqs)] * 2, -1)   # [S, 64]
    sin_full = np.concatenate([np.sin(freqs)] * 2, -1)
    sinx = sin_full.copy()
    sinx[:, :32] *= -1.0

    def tabify(a):  # [S, 64] -> [128, TPB, 64]
        return np.ascontiguousarray(
            a.reshape(TPB, 128, HD).swapaxes(0, 1).astype(np.float32)
        )

    scale = 1.0 / np.sqrt(HD).astype(np.float32)
    cos_q_t = tabify(cos_full * scale)
    sinx_q_t = tabify(sinx * scale)
    cos_k_t = tabify(cos_full)
    sinx_k_t = tabify(sinx)

    kbias_t = ((1.0 - input_mask.astype(np.float32)) * -10000.0).astype(np.float32)

    in_maps = []
    for c in range(NCORES):
        hs = slice(c * HPC * HD, (c + 1) * HPC * HD)     # this core's 128 cols
        wqkv_c = np.ascontiguousarray(
            np.concatenate([wp[:, hs], wp[:, H:][:, hs], wp[:, 2 * H:][:, hs]], axis=1)
        )
        bqkv_c = np.ascontiguousarray(
            np.concatenate([bp[hs], bp[H:][hs], bp[2 * H:][hs]])[None, :]
        )
        in_maps.append({
            "x": x,
            "wqkv": wqkv_c,
            "bqkv": bqkv_c,
            "cos_q": cos_q_t, "sinx_q": sinx_q_t,
            "cos_k": cos_k_t, "sinx_k": sinx_k_t,
            "kbias": kbias_t,
            "ow": attn_ow,
            "ob": np.ascontiguousarray(attn_ob[None, :]),
        })
    return in_maps


_CACHE = {}


def _get_runner():
    """Build nc once and return a callable(in_maps) -> list of out dicts,
    reusing one jitted shard_map across calls."""
    if "runner" in _CACHE:
        return _CACHE["runner"]
    import jax
    import jax.numpy as jnp
    from jax.sharding import Mesh, PartitionSpec
    from jax.experimental.shard_map import shard_map
    from concourse import bass2jax
    from concourse import mybir as _mybir

    nc = build_nc()
    bass2jax.install_neuronx_cc_hook()

    partition_name = nc.partition_id_tensor.name if nc.partition_id_tensor else None
    in_names, out_names, out_avals = [], [], []
    for alloc in nc.m.functions[0].allocations:
        if not isinstance(_mybir.MemoryLocationSet, type) or not isinstance(alloc, _mybir.MemoryLocationSet):
            continue
        name = alloc.memorylocations[0].name
        if alloc.kind == "ExternalInput":
            if name != partition_name:
                in_names.append(name)
        elif alloc.kind == "ExternalOutput":
            out_names.append(name)
            out_avals.append(
                jax.core.ShapedArray(tuple(alloc.tensor_shape), _mybir.dt.np(alloc.dtype))
            )
    n_params = len(in_names)
    all_names = in_names + out_names
    if partition_name is not None:
        all_names.append(partition_name)

    def _body(*args):
        operands = list(args)
        if partition_name is not None:
            operands.append(bass2jax.partition_id_tensor())
        outs = bass2jax._bass_exec_p.bind(
            *operands,
            out_avals=tuple(out_avals),
            in_names=tuple(all_names),
            out_names=tuple(out_names),
            lowering_input_output_aliases=(),
            sim_require_finite=True,
            sim_require_nnan=True,
            nc=nc,
        )
        return tuple(outs)

    devices = jax.devices()[:NCORES]
    mesh = Mesh(np.asarray(devices), ("core",))
    n_outs = len(out_names)
    in_specs = (PartitionSpec("core"),) * (n_params + n_outs)
    out_specs = (PartitionSpec("core"),) * n_outs
    sharded = jax.jit(
        shard_map(_body, mesh=mesh, in_specs=in_specs, out_specs=out_specs,
                  check_rep=False),
        keep_unused=True,
    )

    from jax.sharding import NamedSharding
    shard = NamedSharding(mesh, PartitionSpec("core"))

    def to_device(in_maps):
        concat_in = [
            np.concatenate([np.asarray(in_maps[c][nm]) for c in range(NCORES)], axis=0)
            for nm in in_names
        ]
        concat_zeros = [
            np.zeros((NCORES * a.shape[0], *a.shape[1:]), a.dtype) for a in out_avals
        ]
        return [jax.device_put(a, shard) for a in concat_in + concat_zeros]

    def run_device(dev_args):
        out_arrs = sharded(*dev_args)
        jax.block_until_ready(out_arrs)
        return out_arrs

    def runner(in_maps):
        out_arrs = run_device(to_device(in_maps))
        return [
            {nm: np.asarray(out_arrs[i]).reshape(NCORES, *out_avals[i].shape)[c]
             for i, nm in enumerate(out_names)}
            for c in range(NCORES)
        ]

    runner.to_device = to_device
    runner.run_device = run_device
    _CACHE["runner"] = runner
    return runner


def kernel(**inputs) -> np.ndarray:
    in_maps = make_inputs(**inputs)
    runner = _get_runner()
    results = runner(in_maps)
    full = np.concatenate([results[c]["out_slice"] for c in range(NCORES)], axis=0)
    return full.reshape(B, S, H).astype(np.float32)
